# revision 21
# baseline (speedup 1.0000x reference)
"""Trainium2 Bass kernel for nn_Attention (pooling attention).

Math (per batch b):
    u[b]     = W_score @ h_t[b]            (score = (hidden @ W_score) . h_t
                                            collapses to hidden . (W_score @ h_t))
    score[t] = hidden[b,t,:] . u[b]        (DVE fp16 mul + z1 fold; the second
                                            fold (z2) runs on GpSimd; final
                                            reduce back on DVE, fp16 scores)
    p[t]     = exp(score[t] - 50)          (ScalarE -> bf16, fused accum -> q)
    s[b]     = sum_t p[t]                  (GpSimd partition_all_reduce of q,
                                            written into column b of s_all)
    ctx_u    = sum_t p[t] * hidden[b,t,:]  (PE: bf16 p column as 1-col
                                            stationary vs fp16 y, UNNORMALIZED)
    ctx^T    = scatter of ctx_u rows into persistent PSUM columns
    normalize: ONE reciprocal over s_all at the end; the 1/s scale fuses into
               the ctx^T -> fp16 preT cast on DVE (per-column multiply)
    out[b]   = tanh([ctx/s, h_t[b]] @ W_att)

bf16 p is overflow-safe (fp32-range exponent), so nothing in the per-batch
chain waits on the softmax denominator; unnormalized ctx stays in fp32 PSUM
(max ~1e15 << fp32 range).  The per-batch critical path is just
load -> mul/fold -> exp -> ctx matmuls.

Sharding: data-parallel over batch, 16 batches per core on 8 cores; weights
replicated.  hidden_states is read from HBM exactly once (fp32), cast to fp16
during the DMA (SWDGE cast), and never transposed.

Pipeline design:
  - The y16 load flood starts immediately; ident/ht/wst ride the SWDGE queue
    AHEAD of the flood (they complete in ring order ~10us; anything on the
    sync ring during the flood takes 12..40us to land).  watt stays on the
    sync ring and is only casted right before the epilogue.
  - All u[b]/broadcast work happens on PE+ACT only, in fp16, interleaved with
    the batch loop.
  - First and last batches are loaded and scored in quarter-chunks so the
    pipeline head starts ~6us earlier and the tail overlaps the flood.
  - Engine budget per full batch (vs ~5.3-6us load cadence):
      DVE ~4.8us (mul 2.3, z1 1.3, RED 1.2), GpSimd ~2.5us (descgen, z2,
      all-reduce), ACT ~1.6us (exp, ctx_row, u-chain), PE ~3.5us (16 ctx
      matmuls, scatter, u-chain).
"""

import sys

import numpy as np

_TRN_REPO = "/opt/trn_rl_repo"
if _TRN_REPO not in sys.path:
    sys.path.insert(0, _TRN_REPO)

import concourse.bass as bass
import concourse.bacc as bacc
import concourse.tile as tile
from concourse import mybir
from concourse import bass_isa
from concourse.bass_utils import run_bass_kernel_spmd

N_CORES = 8
B, T, H = 128, 2048, 256
NB = B // N_CORES  # batches per core
P = 128  # SBUF partitions
TT = T // P  # t-tiles per batch
OUT_D = 128
EXP_SHIFT = -50.0  # keeps exp() in fp32/bf16 range; cancels in the softmax ratio

NCH = 4  # first/last batches are loaded/scored in NCH chunks
CTT = TT // NCH
UPRE = 4  # u-chains emitted before the loop; chain b+UPRE emitted in iter b

F32 = mybir.dt.float32
F16 = mybir.dt.float16
BF16 = mybir.dt.bfloat16


def _build_kernel(nc: bass.Bass, tc: "tile.TileContext", hidden, wst, watt, ident, out):
    add = mybir.AluOpType.add

    from contextlib import ExitStack

    with ExitStack() as ctx:
        const = ctx.enter_context(tc.tile_pool(name="const", bufs=1))
        ybufs = ctx.enter_context(tc.tile_pool(name="ybufs", bufs=10))
        sc = ctx.enter_context(tc.tile_pool(name="sc", bufs=3))
        psum_t = ctx.enter_context(tc.tile_pool(name="psum_t", bufs=3, space="PSUM"))
        psum_u = ctx.enter_context(tc.tile_pool(name="psum_u", bufs=2, space="PSUM"))
        psum_p = ctx.enter_context(tc.tile_pool(name="psum_p", bufs=1, space="PSUM"))

        # ---- constants (no DMA needed) -------------------------------------
        ones_row16 = const.tile([1, P], F16, tag="ones_row16")
        nc.vector.memset(ones_row16, 1.0)
        ones_col1 = const.tile([1, 1], F32, tag="ones_col1")
        nc.vector.memset(ones_col1, 1.0)
        shift_col = const.tile([P, 1], F32, tag="shift_col")
        nc.vector.memset(shift_col, EXP_SHIFT)
        s_all = const.tile([P, NB], F32, tag="s_all")  # softmax denominators

        # ---- setup DMAs ----------------------------------------------------
        ident_sb = const.tile([16, 16], F32, tag="ident")
        nc.gpsimd.dma_start(out=ident_sb, in_=ident[:, :])
        ht_sb = const.tile([NB, H], F32, tag="ht")
        nc.gpsimd.dma_start(out=ht_sb, in_=hidden[:, T - 1, :])
        wst_sb = const.tile([P, 2, H], F32, tag="wst")  # W_score^T as [k, kk, h]
        nc.gpsimd.dma_start(out=wst_sb, in_=wst.rearrange("(kk p) h -> p kk h", p=P))
        watt_sb = const.tile([P, 4, OUT_D], F32, tag="watt")  # W_att as [d, dd, j]
        nc.sync.dma_start(out=watt_sb, in_=watt.rearrange("(dd p) j -> p dd j", p=P))

        # ---- y16 load flood (SWDGE cast fp32->fp16) ------------------------
        # descgen for load k is emitted a few batches ahead of its consumer;
        # emitting ALL loads upfront would queue the per-batch gpsimd compute
        # (z2 fold, all-reduce) behind buffer-WAR-blocked descgens and
        # serialize the whole pipeline.
        CHUNKED = (0, NB - 1)
        PREF = 5
        ylist = {}
        ychunks = {}

        def emit_load_full(k):
            y = ybufs.tile([P, TT, H], F16, tag="y16", name=f"y16_{k}")
            nc.gpsimd.dma_start(
                out=y, in_=hidden[k].rearrange("(p i) h -> p i h", i=TT)
            )
            ylist[k] = y

        def emit_load_chunk(k, c):
            hk = hidden[k].rearrange("(p i) h -> p i h", i=TT)
            yc = ybufs.tile([P, CTT, H], F16, tag="y16c", name=f"y16c_{k}_{c}")
            nc.gpsimd.dma_start(out=yc, in_=hk[:, c * CTT : (c + 1) * CTT, :])
            ychunks.setdefault(k, []).append(yc)

        for c in range(NCH):
            emit_load_chunk(0, c)
        for k in range(1, PREF + 1):
            emit_load_full(k)

        # ---- h_t^T (fp16) and fp16 copy of W_score^T -----------------------
        htT16 = const.tile([P, 2, NB], F16, tag="htT16")  # h_t^T halves [k, half, b]
        for half in range(2):
            ps_tr = psum_t.tile([P, NB], F32, tag="ptmp", name=f"ps_tr{half}")
            nc.tensor.matmul(
                ps_tr,
                lhsT=ht_sb[:, half * P : (half + 1) * P],
                rhs=ident_sb,
                start=True,
                stop=True,
            )
            nc.scalar.copy(out=htT16[:, half, :], in_=ps_tr)
        wst16 = const.tile([P, 2, H], F16, tag="wst16")
        nc.scalar.copy(out=wst16, in_=wst_sb)

        # u[b] = h_t[b] @ W_score^T via M=1 fp16 matmuls; broadcast via a K=1
        # matmul.  No DMAs -> nothing contends with the flood.
        ubc_all = const.tile([P, NB, H], F16, tag="ubc_all")

        def emit_uchain(b):
            ps_ub = psum_u.tile([1, H], F32, tag="pub", name=f"pub{b}")
            for half in range(2):
                nc.tensor.matmul(
                    ps_ub,
                    lhsT=htT16[:, half, b : b + 1],
                    rhs=wst16[:, half, :],
                    start=(half == 0),
                    stop=(half == 1),
                )
            u16b = sc.tile([1, H], F16, tag="u16b", name=f"u16b{b}")
            nc.scalar.copy(out=u16b, in_=ps_ub)
            ps_ubc = psum_t.tile([P, H], F32, tag="ptmp", name=f"pubc{b}")
            nc.tensor.matmul(ps_ubc, lhsT=ones_row16, rhs=u16b, start=True, stop=True)
            nc.scalar.copy(out=ubc_all[:, b, :], in_=ps_ubc)

        for b in range(UPRE + 1):  # loop below starts at b=1, so chains 0..4 here
            emit_uchain(b)

        # ---- persistent PSUM accumulators for ctx^T ------------------------
        ctxT_ps = [
            psum_p.tile([P, NB], F32, tag=f"ctxT{j}", name=f"ctxT{j}")
            for j in range(2)
        ]

        state = {}  # batch -> dict of tiles needed by the deferred stages

        def ubc_rep(b, rep):
            ubc = ubc_all[:, b, :]
            return bass.AP(
                tensor=ubc.tensor,
                offset=ubc.offset,
                ap=[list(ubc.ap[0]), [0, rep], list(ubc.ap[1])],
            )

        def emit_mul_z1(b, y, nt, tag_sfx):
            z = sc.tile([P, nt, H], F16, tag="z" + tag_sfx)
            nc.vector.tensor_mul(z, y, ubc_rep(b, nt))
            z1 = sc.tile([P, nt, 128], F16, tag="z1" + tag_sfx)
            nc.vector.tensor_add(z1, z[:, :, 0:128], z[:, :, 128:256])
            return z1

        def emit_z2(z1, nt, tag_sfx):
            # second fold runs on GpSimd (otherwise idle) to unload the DVE
            z2 = sc.tile([P, nt, 64], F16, tag="z2" + tag_sfx)
            nc.gpsimd.tensor_add(z2, z1[:, :, 0:64], z1[:, :, 64:128])
            return z2

        def emit_red(z2, score_sl):
            with nc.allow_low_precision(reason="fp16 softmax scores"):
                nc.vector.tensor_reduce(
                    out=score_sl, in_=z2, axis=mybir.AxisListType.X, op=add
                )

        def emit_exp(score_sl, p_sl, q_sl):
            nc.scalar.activation(
                out=p_sl,
                in_=score_sl,
                func=mybir.ActivationFunctionType.Exp,
                bias=shift_col,
                scale=1.0,
                accum_out=q_sl,
            )

        def emit_ared(b):
            # s[b] (broadcast over partitions) -> column b of s_all
            nc.gpsimd.partition_all_reduce(
                s_all[:, b : b + 1], state[b]["q"], P, bass_isa.ReduceOp.add
            )

        def emit_ctx(b):
            p_t = state[b]["p"]
            ctx_ps = psum_t.tile([1, H], F32, tag="ptmp", name=f"ctx{b}")
            y16 = ylist.pop(b)
            for i in range(TT):
                nc.tensor.matmul(
                    ctx_ps,
                    lhsT=p_t[:, i : i + 1],
                    rhs=y16[:, i, :],
                    start=(i == 0),
                    stop=(i == TT - 1),
                )
            state[b]["ctx_ps"] = ctx_ps

        def emit_ctx_row(b):
            ctx_row = sc.tile([1, H], F32, tag="ctx_row")
            nc.scalar.copy(out=ctx_row, in_=state[b]["ctx_ps"])
            state[b]["ctx_row"] = ctx_row

        def emit_scatter(b):
            ctx_row = state[b]["ctx_row"]
            for j in range(2):
                nc.tensor.matmul(
                    ctxT_ps[j][:, b : b + 1],
                    lhsT=ctx_row[:, j * P : (j + 1) * P],
                    rhs=ones_col1,
                    start=True,
                    stop=True,
                )
            del state[b]

        def emit_chunked(b, after_first_mul=None):
            # quarter-chunk pipeline: mul/fold/exp/ctx per chunk; the softmax
            # denominator is assembled once after the last chunk
            chunks = ychunks[b]
            score = sc.tile([P, TT], F16, tag="score", name=f"score{b}")
            p_t = sc.tile([P, TT], BF16, tag="p", name=f"p{b}")
            q4 = sc.tile([P, NCH], F32, tag="q4", name=f"q4_{b}")
            ctx_ps = psum_t.tile([1, H], F32, tag="ptmp", name=f"ctxc{b}")
            state[b] = {"p": p_t}
            z1s = {}
            for c in range(NCH):
                z1s[c] = emit_mul_z1(b, chunks[c], CTT, "c")
                if c == 0 and after_first_mul is not None:
                    after_first_mul()
                z2 = emit_z2(z1s[c], CTT, "c")
                sl = slice(c * CTT, (c + 1) * CTT)
                emit_red(z2, score[:, sl])
                emit_exp(score[:, sl], p_t[:, sl], q4[:, c : c + 1])
                for i in range(c * CTT, (c + 1) * CTT):
                    nc.tensor.matmul(
                        ctx_ps,
                        lhsT=p_t[:, i : i + 1],
                        rhs=chunks[c][:, i % CTT, :],
                        start=(i == 0),
                        stop=(i == TT - 1),
                    )
            state[b]["ctx_ps"] = ctx_ps
            qs = sc.tile([P, 1], F32, tag="qs", name=f"qs{b}")
            nc.vector.tensor_reduce(out=qs, in_=q4, axis=mybir.AxisListType.X, op=add)
            state[b]["q"] = qs
            emit_ared(b)

        # ---- batch 0: chunked so the pipeline head starts ~6us earlier -----
        emit_chunked(0)

        # ---- full batches 1..14 (stages offset to hide cross-engine hops) --
        pend = {}  # b -> z2 tile awaiting its reduce
        for b in range(1, NB - 1):
            k = b + PREF
            if k <= NB - 2:
                emit_load_full(k)
            elif k <= NB - 2 + NCH:
                emit_load_chunk(NB - 1, k - (NB - 1))
            if b + UPRE < NB:
                emit_uchain(b + UPRE)
            z1 = emit_mul_z1(b, ylist[b], TT, "")
            if b - 1 in pend:  # reduce of the previous batch (z2 on gpsimd done)
                bp = b - 1
                score = sc.tile([P, TT], F16, tag="score")
                emit_red(pend.pop(bp), score)
                p_t = sc.tile([P, TT], BF16, tag="p")
                q = sc.tile([P, 1], F32, tag="q")
                state[bp] = {"p": p_t, "q": q}
                emit_exp(score, p_t, q)
            pend[b] = emit_z2(z1, TT, "")
            if b - 1 >= 1 and b - 1 not in CHUNKED:
                emit_ared(b - 1)
                emit_ctx(b - 1)
            if b - 2 >= 0 and (b - 2) in state and "ctx_ps" in state[b - 2]:
                emit_ctx_row(b - 2)
                emit_scatter(b - 2)

        # ---- drain b14, then the chunked last batch ------------------------
        def _finish_b14():
            bp = NB - 2
            score = sc.tile([P, TT], F16, tag="score")
            emit_red(pend.pop(bp), score)
            p_t = sc.tile([P, TT], BF16, tag="p")
            q = sc.tile([P, 1], F32, tag="q")
            state[bp] = {"p": p_t, "q": q}
            emit_exp(score, p_t, q)
            emit_ared(bp)
            emit_ctx(bp)
            emit_ctx_row(NB - 3)
            emit_scatter(NB - 3)

        emit_chunked(NB - 1, after_first_mul=_finish_b14)
        emit_ctx_row(NB - 2)
        emit_scatter(NB - 2)
        emit_ctx_row(NB - 1)
        emit_scatter(NB - 1)

        # ---- finalize: 1/s, concat with h_t, @W_att, tanh ------------------
        rs_all = sc.tile([P, NB], F32, tag="rs_all")
        nc.vector.reciprocal(out=rs_all, in_=s_all)
        # watt16 cast sits here so its wait on the (slow, sync-ring) watt DMA
        # never head-of-line blocks the per-batch ACT stream
        watt16 = const.tile([P, 4, OUT_D], F16, tag="watt16")
        nc.scalar.copy(out=watt16, in_=watt_sb)
        preT = sc.tile([P, 2, NB], F16, tag="preT")
        with nc.allow_low_precision(reason="normalized ctx fits fp16"):
            for j in range(2):
                nc.vector.tensor_mul(preT[:, j, :], ctxT_ps[j], rs_all)

        out_ps = psum_t.tile([NB, OUT_D], F32, tag="ptmp")
        for dd in range(4):
            lhsT = preT[:, dd, :] if dd < 2 else htT16[:, dd - 2, :]
            nc.tensor.matmul(
                out_ps,
                lhsT=lhsT,
                rhs=watt16[:, dd, :],
                start=(dd == 0),
                stop=(dd == 3),
            )
        out_sb = sc.tile([NB, OUT_D], F32, tag="out_sb")
        nc.scalar.activation(
            out=out_sb, in_=out_ps, func=mybir.ActivationFunctionType.Tanh
        )
        nc.sync.dma_start(out=out[:, :], in_=out_sb)


_NC_CACHE = {}


def _get_nc():
    if "nc" not in _NC_CACHE:
        nc = bacc.Bacc("TRN2", target_bir_lowering=False, debug=False)
        hidden = nc.declare_dram_parameter("hidden", [NB, T, H], F32, isOutput=False)
        wst = nc.declare_dram_parameter("w_score_t", [H, H], F32, isOutput=False)
        watt = nc.declare_dram_parameter("w_att", [2 * H, OUT_D], F32, isOutput=False)
        ident = nc.declare_dram_parameter("ident16", [16, 16], F32, isOutput=False)
        out = nc.declare_dram_parameter("out", [NB, OUT_D], F32, isOutput=True)
        with tile.TileContext(nc) as tc:
            _build_kernel(nc, tc, hidden, wst, watt, ident, out)
        nc.compile()
        _NC_CACHE["nc"] = nc
    return _NC_CACHE["nc"]


def _run(hidden_states, W_score, W_att, trace=False, trace_kwargs=None):
    hidden_states = np.ascontiguousarray(np.asarray(hidden_states, dtype=np.float32))
    W_score = np.asarray(W_score, dtype=np.float32)
    W_att = np.ascontiguousarray(np.asarray(W_att, dtype=np.float32))
    wst = np.ascontiguousarray(W_score.T)
    ident = np.eye(16, dtype=np.float32)

    nc = _get_nc()
    in_maps = []
    for c in range(N_CORES):
        in_maps.append(
            {
                "hidden": hidden_states[c * NB : (c + 1) * NB],
                "w_score_t": wst,
                "w_att": W_att,
                "ident16": ident,
            }
        )
    kwargs = {}
    if trace:
        kwargs["trace"] = True
        if trace_kwargs:
            kwargs.update(trace_kwargs)
    res = run_bass_kernel_spmd(nc, in_maps, list(range(N_CORES)), **kwargs)
    out = np.concatenate([res.results[c]["out"] for c in range(N_CORES)], axis=0)
    return out, res


def kernel(hidden_states, W_score, W_att):
    out, _ = _run(hidden_states, W_score, W_att, trace=False)
    return out


# revision 22
# speedup vs baseline: 2.6380x; 2.6380x over previous
"""Trainium2 Bass kernel for nn_Attention (pooling attention).

Math (per batch b):
    u[b]     = W_score @ h_t[b]            (score = (hidden @ W_score) . h_t
                                            collapses to hidden . (W_score @ h_t))
    score[t] = hidden[b,t,:] . u[b]        (DVE fp16 mul + pairwise tree,
                                            fp16 scores)
    p[t]     = exp(score[t] - 50)          (ScalarE -> bf16, fused accum -> q)
    s[b]     = sum_t p[t]                  (PE ones-matmul of q into column b
                                            of a persistent PSUM tile)
    ctx_u    = sum_t p[t] * hidden[b,t,:]  (PE: bf16 p column as 1-col
                                            stationary vs fp16 y, UNNORMALIZED)
    ctx^T    = scatter of ctx_u rows into persistent PSUM columns
    normalize: ONE reciprocal over s_all at the end; the 1/s scale fuses into
               the ctx^T -> fp16 preT cast on DVE (per-column multiply)
    out[b]   = tanh([ctx/s, h_t[b]] @ W_att)

bf16 p is overflow-safe (fp32-range exponent), so nothing in the per-batch
chain waits on the softmax denominator; unnormalized ctx stays in fp32 PSUM
(max ~1e15 << fp32 range).  The DVE stream is a pure load-gated streak
(mul/z1/z2/reduce), with no cross-engine waits.

Sharding: data-parallel over batch, 16 batches per core on 8 cores; weights
replicated.  hidden_states is read from HBM exactly once (fp32), cast to fp16
during the DMA (SWDGE cast), and never transposed.

Pipeline design:
  - GpSimd runs ONLY the SWDGE descriptor generation (any compute op on its
    FIFO couples the score chain to buffer-WAR-blocked descgens and
    serializes the pipeline -- measured 2.5x blowup).
  - The y16 load flood starts immediately; ident/ht/wst ride the SWDGE queue
    AHEAD of the flood (they complete in ring order ~10us; anything on the
    sync ring during the flood takes 12..40us to land).  watt stays on the
    sync ring and is only casted right before the epilogue.
  - All u[b]/broadcast work happens on PE+ACT only, in fp16, interleaved with
    the batch loop.
  - First and last batches are loaded and scored in quarter-chunks so the
    pipeline head starts ~6us earlier and the tail overlaps the flood.
"""

import sys

import numpy as np

_TRN_REPO = "/opt/trn_rl_repo"
if _TRN_REPO not in sys.path:
    sys.path.insert(0, _TRN_REPO)

import concourse.bass as bass
import concourse.bacc as bacc
import concourse.tile as tile
from concourse import mybir
from concourse.bass_utils import run_bass_kernel_spmd

N_CORES = 8
B, T, H = 128, 2048, 256
NB = B // N_CORES  # batches per core
P = 128  # SBUF partitions
TT = T // P  # t-tiles per batch
OUT_D = 128
EXP_SHIFT = -50.0  # keeps exp() in fp32/bf16 range; cancels in the softmax ratio

NCH = 4  # first/last batches are loaded/scored in NCH chunks
CTT = TT // NCH
UPRE = 4  # u-chains emitted before the loop; chain b+UPRE emitted in iter b

F32 = mybir.dt.float32
F16 = mybir.dt.float16
BF16 = mybir.dt.bfloat16


def _build_kernel(nc: bass.Bass, tc: "tile.TileContext", hidden, wst, watt, ident, out):
    add = mybir.AluOpType.add

    from contextlib import ExitStack

    with ExitStack() as ctx:
        const = ctx.enter_context(tc.tile_pool(name="const", bufs=1))
        ybufs = ctx.enter_context(tc.tile_pool(name="ybufs", bufs=10))
        sc = ctx.enter_context(tc.tile_pool(name="sc", bufs=3))
        psum_t = ctx.enter_context(tc.tile_pool(name="psum_t", bufs=3, space="PSUM"))
        psum_u = ctx.enter_context(tc.tile_pool(name="psum_u", bufs=2, space="PSUM"))
        psum_p = ctx.enter_context(tc.tile_pool(name="psum_p", bufs=1, space="PSUM"))

        # ---- constants (no DMA needed) -------------------------------------
        ones_row16 = const.tile([1, P], F16, tag="ones_row16")
        nc.vector.memset(ones_row16, 1.0)
        ones128 = const.tile([P, P], F32, tag="ones128")
        nc.vector.memset(ones128, 1.0)
        ones_col1 = const.tile([1, 1], F32, tag="ones_col1")
        nc.vector.memset(ones_col1, 1.0)
        shift_col = const.tile([P, 1], F32, tag="shift_col")
        nc.vector.memset(shift_col, EXP_SHIFT)

        # ---- setup DMAs ----------------------------------------------------
        ident_sb = const.tile([16, 16], F32, tag="ident")
        nc.gpsimd.dma_start(out=ident_sb, in_=ident[:, :])
        ht_sb = const.tile([NB, H], F32, tag="ht")
        nc.gpsimd.dma_start(out=ht_sb, in_=hidden[:, T - 1, :])
        wst_sb = const.tile([P, 2, H], F32, tag="wst")  # W_score^T as [k, kk, h]
        nc.gpsimd.dma_start(out=wst_sb, in_=wst.rearrange("(kk p) h -> p kk h", p=P))
        watt_sb = const.tile([P, 4, OUT_D], F32, tag="watt")  # W_att as [d, dd, j]
        nc.sync.dma_start(out=watt_sb, in_=watt.rearrange("(dd p) j -> p dd j", p=P))

        # ---- y16 load flood (SWDGE cast fp32->fp16), starts immediately ----
        CHUNKED = (0, NB - 1)
        ylist = {}
        ychunks = {}
        for k in range(NB):
            if k in CHUNKED:
                hk = hidden[k].rearrange("(p i) h -> p i h", i=TT)
                tiles = []
                for c in range(NCH):
                    yc = ybufs.tile([P, CTT, H], F16, tag="y16c", name=f"y16c_{k}_{c}")
                    nc.gpsimd.dma_start(out=yc, in_=hk[:, c * CTT : (c + 1) * CTT, :])
                    tiles.append(yc)
                ychunks[k] = tiles
            else:
                y = ybufs.tile([P, TT, H], F16, tag="y16", name=f"y16_{k}")
                nc.gpsimd.dma_start(
                    out=y, in_=hidden[k].rearrange("(p i) h -> p i h", i=TT)
                )
                ylist[k] = y

        # ---- h_t^T (fp16) and fp16 copy of W_score^T -----------------------
        htT16 = const.tile([P, 2, NB], F16, tag="htT16")  # h_t^T halves [k, half, b]
        for half in range(2):
            ps_tr = psum_t.tile([P, NB], F32, tag="ptmp", name=f"ps_tr{half}")
            nc.tensor.matmul(
                ps_tr,
                lhsT=ht_sb[:, half * P : (half + 1) * P],
                rhs=ident_sb,
                start=True,
                stop=True,
            )
            nc.scalar.copy(out=htT16[:, half, :], in_=ps_tr)
        wst16 = const.tile([P, 2, H], F16, tag="wst16")
        nc.scalar.copy(out=wst16, in_=wst_sb)

        # u[b] = h_t[b] @ W_score^T via M=1 fp16 matmuls; broadcast via a K=1
        # matmul.  No DMAs -> nothing contends with the flood.
        ubc_all = const.tile([P, NB, H], F16, tag="ubc_all")

        def emit_uchain(b):
            ps_ub = psum_u.tile([1, H], F32, tag="pub", name=f"pub{b}")
            for half in range(2):
                nc.tensor.matmul(
                    ps_ub,
                    lhsT=htT16[:, half, b : b + 1],
                    rhs=wst16[:, half, :],
                    start=(half == 0),
                    stop=(half == 1),
                )
            u16b = sc.tile([1, H], F16, tag="u16b", name=f"u16b{b}")
            nc.scalar.copy(out=u16b, in_=ps_ub)
            ps_ubc = psum_t.tile([P, H], F32, tag="ptmp", name=f"pubc{b}")
            nc.tensor.matmul(ps_ubc, lhsT=ones_row16, rhs=u16b, start=True, stop=True)
            nc.scalar.copy(out=ubc_all[:, b, :], in_=ps_ubc)

        for b in range(UPRE + 1):  # loop below starts at b=1, so chains 0..4 here
            emit_uchain(b)

        # ---- persistent PSUM accumulators: ctx^T and softmax sums ----------
        ctxT_ps = [
            psum_p.tile([P, NB], F32, tag=f"ctxT{j}", name=f"ctxT{j}")
            for j in range(2)
        ]
        s_all_ps = psum_p.tile([P, NB], F32, tag="s_all", name="s_all")

        state = {}  # batch -> dict of tiles needed by the deferred stages

        def ubc_rep(b, rep):
            ubc = ubc_all[:, b, :]
            return bass.AP(
                tensor=ubc.tensor,
                offset=ubc.offset,
                ap=[list(ubc.ap[0]), [0, rep], list(ubc.ap[1])],
            )

        def dve_tree(b, y, score_sl, nt, tag_sfx):
            z = sc.tile([P, nt, H], F16, tag="z" + tag_sfx)
            nc.vector.tensor_mul(z, y, ubc_rep(b, nt))
            z1 = sc.tile([P, nt, 128], F16, tag="z1" + tag_sfx)
            nc.vector.tensor_add(z1, z[:, :, 0:128], z[:, :, 128:256])
            z2 = sc.tile([P, nt, 64], F16, tag="z2" + tag_sfx)
            nc.vector.tensor_add(z2, z1[:, :, 0:64], z1[:, :, 64:128])
            with nc.allow_low_precision(reason="fp16 softmax scores"):
                nc.vector.tensor_reduce(
                    out=score_sl, in_=z2, axis=mybir.AxisListType.X, op=add
                )

        def emit_exp(score_sl, p_sl, q_sl):
            nc.scalar.activation(
                out=p_sl,
                in_=score_sl,
                func=mybir.ActivationFunctionType.Exp,
                bias=shift_col,
                scale=1.0,
                accum_out=q_sl,
            )

        def emit_s(b, q_sl, start, stop):
            nc.tensor.matmul(
                s_all_ps[:, b : b + 1], lhsT=ones128, rhs=q_sl, start=start, stop=stop
            )

        def emit_ctx(b):
            p_t = state[b]["p"]
            ctx_ps = psum_t.tile([1, H], F32, tag="ptmp", name=f"ctx{b}")
            y16 = ylist.pop(b)
            for i in range(TT):
                nc.tensor.matmul(
                    ctx_ps,
                    lhsT=p_t[:, i : i + 1],
                    rhs=y16[:, i, :],
                    start=(i == 0),
                    stop=(i == TT - 1),
                )
            state[b]["ctx_ps"] = ctx_ps

        def emit_ctx_row(b):
            ctx_row = sc.tile([1, H], F32, tag="ctx_row")
            nc.scalar.copy(out=ctx_row, in_=state[b]["ctx_ps"])
            state[b]["ctx_row"] = ctx_row

        def emit_scatter(b):
            ctx_row = state[b]["ctx_row"]
            for j in range(2):
                nc.tensor.matmul(
                    ctxT_ps[j][:, b : b + 1],
                    lhsT=ctx_row[:, j * P : (j + 1) * P],
                    rhs=ones_col1,
                    start=True,
                    stop=True,
                )
            del state[b]

        def emit_chunked(b, after_first_mul=None):
            # quarter-chunk pipeline: tree/exp/ctx/s per chunk; s accumulates
            # across chunks in PSUM (bf16 p needs no pre-normalization)
            chunks = ychunks[b]
            score = sc.tile([P, TT], F16, tag="score", name=f"score{b}")
            p_t = sc.tile([P, TT], BF16, tag="p", name=f"p{b}")
            q4 = sc.tile([P, NCH], F32, tag="q4", name=f"q4_{b}")
            ctx_ps = psum_t.tile([1, H], F32, tag="ptmp", name=f"ctxc{b}")
            state[b] = {"p": p_t}
            for c in range(NCH):
                sl = slice(c * CTT, (c + 1) * CTT)
                if c == 0 and after_first_mul is not None:
                    zc = sc.tile([P, CTT, H], F16, tag="zc")
                    nc.vector.tensor_mul(zc, chunks[c], ubc_rep(b, CTT))
                    after_first_mul()
                    z1 = sc.tile([P, CTT, 128], F16, tag="z1c")
                    nc.vector.tensor_add(z1, zc[:, :, 0:128], zc[:, :, 128:256])
                    z2 = sc.tile([P, CTT, 64], F16, tag="z2c")
                    nc.vector.tensor_add(z2, z1[:, :, 0:64], z1[:, :, 64:128])
                    with nc.allow_low_precision(reason="fp16 softmax scores"):
                        nc.vector.tensor_reduce(
                            out=score[:, sl], in_=z2, axis=mybir.AxisListType.X, op=add
                        )
                else:
                    dve_tree(b, chunks[c], score[:, sl], CTT, "c")
                emit_exp(score[:, sl], p_t[:, sl], q4[:, c : c + 1])
                for i in range(c * CTT, (c + 1) * CTT):
                    nc.tensor.matmul(
                        ctx_ps,
                        lhsT=p_t[:, i : i + 1],
                        rhs=chunks[c][:, i % CTT, :],
                        start=(i == 0),
                        stop=(i == TT - 1),
                    )
            state[b]["ctx_ps"] = ctx_ps
            for c in range(NCH):
                emit_s(b, q4[:, c : c + 1], start=(c == 0), stop=(c == NCH - 1))

        # ---- batch 0: chunked so the pipeline head starts ~6us earlier -----
        emit_chunked(0)

        # ---- full batches 1..14 --------------------------------------------
        for b in range(1, NB - 1):
            if b + UPRE < NB:
                emit_uchain(b + UPRE)
            score = sc.tile([P, TT], F16, tag="score")
            # mul first, then ctx(b-1) (PE) so the PE work for the previous
            # batch is emitted while the DVE chews on this batch
            z = sc.tile([P, TT, H], F16, tag="z")
            nc.vector.tensor_mul(z, ylist[b], ubc_rep(b, TT))
            if b - 1 not in CHUNKED:
                emit_ctx(b - 1)
            z1 = sc.tile([P, TT, 128], F16, tag="z1")
            nc.vector.tensor_add(z1, z[:, :, 0:128], z[:, :, 128:256])
            z2 = sc.tile([P, TT, 64], F16, tag="z2")
            nc.vector.tensor_add(z2, z1[:, :, 0:64], z1[:, :, 64:128])
            with nc.allow_low_precision(reason="fp16 softmax scores"):
                nc.vector.tensor_reduce(
                    out=score, in_=z2, axis=mybir.AxisListType.X, op=add
                )
            p_t = sc.tile([P, TT], BF16, tag="p")
            q = sc.tile([P, 1], F32, tag="q")
            state[b] = {"p": p_t}
            emit_exp(score, p_t, q)
            emit_s(b, q, start=True, stop=True)
            emit_ctx_row(b - 1)
            emit_scatter(b - 1)

        # ---- last batch: chunked (tail overlaps the flood) -----------------
        def _finish_b14():
            emit_ctx(NB - 2)
            emit_ctx_row(NB - 2)
            emit_scatter(NB - 2)

        emit_chunked(NB - 1, after_first_mul=_finish_b14)
        emit_ctx_row(NB - 1)
        emit_scatter(NB - 1)

        # ---- finalize: 1/s, concat with h_t, @W_att, tanh ------------------
        rs_all = sc.tile([P, NB], F32, tag="rs_all")
        nc.vector.reciprocal(out=rs_all, in_=s_all_ps)
        # watt16 cast sits here so its wait on the (slow, sync-ring) watt DMA
        # never head-of-line blocks the per-batch ACT stream
        watt16 = const.tile([P, 4, OUT_D], F16, tag="watt16")
        nc.scalar.copy(out=watt16, in_=watt_sb)
        preT = sc.tile([P, 2, NB], F16, tag="preT")
        with nc.allow_low_precision(reason="normalized ctx fits fp16"):
            for j in range(2):
                nc.vector.tensor_mul(preT[:, j, :], ctxT_ps[j], rs_all)

        out_ps = psum_t.tile([NB, OUT_D], F32, tag="ptmp")
        for dd in range(4):
            lhsT = preT[:, dd, :] if dd < 2 else htT16[:, dd - 2, :]
            nc.tensor.matmul(
                out_ps,
                lhsT=lhsT,
                rhs=watt16[:, dd, :],
                start=(dd == 0),
                stop=(dd == 3),
            )
        out_sb = sc.tile([NB, OUT_D], F32, tag="out_sb")
        nc.scalar.activation(
            out=out_sb, in_=out_ps, func=mybir.ActivationFunctionType.Tanh
        )
        nc.sync.dma_start(out=out[:, :], in_=out_sb)


_NC_CACHE = {}


def _get_nc():
    if "nc" not in _NC_CACHE:
        nc = bacc.Bacc("TRN2", target_bir_lowering=False, debug=False)
        hidden = nc.declare_dram_parameter("hidden", [NB, T, H], F32, isOutput=False)
        wst = nc.declare_dram_parameter("w_score_t", [H, H], F32, isOutput=False)
        watt = nc.declare_dram_parameter("w_att", [2 * H, OUT_D], F32, isOutput=False)
        ident = nc.declare_dram_parameter("ident16", [16, 16], F32, isOutput=False)
        out = nc.declare_dram_parameter("out", [NB, OUT_D], F32, isOutput=True)
        with tile.TileContext(nc) as tc:
            _build_kernel(nc, tc, hidden, wst, watt, ident, out)
        nc.compile()
        _NC_CACHE["nc"] = nc
    return _NC_CACHE["nc"]


def _run(hidden_states, W_score, W_att, trace=False, trace_kwargs=None):
    hidden_states = np.ascontiguousarray(np.asarray(hidden_states, dtype=np.float32))
    W_score = np.asarray(W_score, dtype=np.float32)
    W_att = np.ascontiguousarray(np.asarray(W_att, dtype=np.float32))
    wst = np.ascontiguousarray(W_score.T)
    ident = np.eye(16, dtype=np.float32)

    nc = _get_nc()
    in_maps = []
    for c in range(N_CORES):
        in_maps.append(
            {
                "hidden": hidden_states[c * NB : (c + 1) * NB],
                "w_score_t": wst,
                "w_att": W_att,
                "ident16": ident,
            }
        )
    kwargs = {}
    if trace:
        kwargs["trace"] = True
        if trace_kwargs:
            kwargs.update(trace_kwargs)
    res = run_bass_kernel_spmd(nc, in_maps, list(range(N_CORES)), **kwargs)
    out = np.concatenate([res.results[c]["out"] for c in range(N_CORES)], axis=0)
    return out, res


def kernel(hidden_states, W_score, W_att):
    out, _ = _run(hidden_states, W_score, W_att, trace=False)
    return out


# revision 26
# speedup vs baseline: 2.6608x; 1.0086x over previous
"""Trainium2 Bass kernel for nn_Attention (pooling attention).

Math (per batch b):
    u[b]     = W_score @ h_t[b]            (score = (hidden @ W_score) . h_t
                                            collapses to hidden . (W_score @ h_t))
    score[t] = hidden[b,t,:] . u[b]        (DVE fp16 mul + pairwise tree,
                                            fp16 scores)
    p[t]     = exp(score[t] - 50)          (ScalarE -> bf16, fused accum -> q)
    s[b]     = sum_t p[t]                  (PE ones-matmul of q into column b
                                            of a persistent PSUM tile)
    ctx_u    = sum_t p[t] * hidden[b,t,:]  (PE: bf16 p column as 1-col
                                            stationary vs fp16 y, UNNORMALIZED)
    ctx^T    = scatter of ctx_u rows into persistent PSUM columns
    normalize: ONE reciprocal over s_all at the end; the 1/s scale fuses into
               the ctx^T -> fp16 preT cast on DVE (per-column multiply)
    out[b]   = tanh([ctx/s, h_t[b]] @ W_att)

bf16 p is overflow-safe (fp32-range exponent), so nothing in the per-batch
chain waits on the softmax denominator; unnormalized ctx stays in fp32 PSUM
(max ~1e15 << fp32 range).  The DVE stream is a pure load-gated streak
(mul/z1/z2/reduce), with no cross-engine waits.

Sharding: data-parallel over batch, 16 batches per core on 8 cores; weights
replicated.  hidden_states is read from HBM exactly once (fp32), cast to fp16
during the DMA (SWDGE cast), and never transposed.

Pipeline design:
  - GpSimd runs ONLY the SWDGE descriptor generation (any compute op on its
    FIFO couples the score chain to buffer-WAR-blocked descgens and
    serializes the pipeline -- measured 2.5x blowup).
  - The y16 load flood starts immediately; ident/ht/wst ride the SWDGE queue
    AHEAD of the flood (they complete in ring order ~10us; anything on the
    sync ring during the flood takes 12..40us to land).  watt stays on the
    sync ring and is only casted right before the epilogue.
  - All u[b]/broadcast work happens on PE+ACT only, in fp16, interleaved with
    the batch loop.
  - First and last batches are loaded and scored in quarter-chunks so the
    pipeline head starts ~6us earlier and the tail overlaps the flood.
"""

import sys

import numpy as np

_TRN_REPO = "/opt/trn_rl_repo"
if _TRN_REPO not in sys.path:
    sys.path.insert(0, _TRN_REPO)

import concourse.bass as bass
import concourse.bacc as bacc
import concourse.tile as tile
from concourse import mybir
from concourse.bass_utils import run_bass_kernel_spmd

N_CORES = 8
B, T, H = 128, 2048, 256
NB = B // N_CORES  # batches per core
P = 128  # SBUF partitions
TT = T // P  # t-tiles per batch
OUT_D = 128
EXP_SHIFT = -50.0  # keeps exp() in fp32/bf16 range; cancels in the softmax ratio

NCH = 4  # first/last batches are loaded/scored in NCH chunks
CTT = TT // NCH
UPRE = 4  # u-chains emitted before the loop; chain b+UPRE emitted in iter b

F32 = mybir.dt.float32
F16 = mybir.dt.float16
BF16 = mybir.dt.bfloat16


def _build_kernel(nc: bass.Bass, tc: "tile.TileContext", hidden, wst, watt, ident, out):
    add = mybir.AluOpType.add

    from contextlib import ExitStack

    with ExitStack() as ctx:
        const = ctx.enter_context(tc.tile_pool(name="const", bufs=1))
        ybufs = ctx.enter_context(tc.tile_pool(name="ybufs", bufs=10))
        sc = ctx.enter_context(tc.tile_pool(name="sc", bufs=3))
        psum_t = ctx.enter_context(tc.tile_pool(name="psum_t", bufs=3, space="PSUM"))
        psum_u = ctx.enter_context(tc.tile_pool(name="psum_u", bufs=2, space="PSUM"))
        psum_p = ctx.enter_context(tc.tile_pool(name="psum_p", bufs=1, space="PSUM"))

        # ---- constants (no DMA needed) -------------------------------------
        ones_row16 = const.tile([1, P], F16, tag="ones_row16")
        nc.vector.memset(ones_row16, 1.0)
        ones128_16 = const.tile([P, P], BF16, tag="ones128_16")
        nc.vector.memset(ones128_16, 1.0)
        ones_col1 = const.tile([1, 1], F32, tag="ones_col1")
        nc.vector.memset(ones_col1, 1.0)
        shift_col = const.tile([P, 1], F32, tag="shift_col")
        nc.vector.memset(shift_col, EXP_SHIFT)

        # ---- setup DMAs ----------------------------------------------------
        ident_sb = const.tile([16, 16], F32, tag="ident")
        nc.gpsimd.dma_start(out=ident_sb, in_=ident[:, :])
        ht_sb = const.tile([NB, H], F32, tag="ht")
        nc.gpsimd.dma_start(out=ht_sb, in_=hidden[:, T - 1, :])
        wst_sb = const.tile([P, 2, H], F32, tag="wst")  # W_score^T as [k, kk, h]
        nc.gpsimd.dma_start(out=wst_sb, in_=wst.rearrange("(kk p) h -> p kk h", p=P))
        watt_sb = const.tile([P, 4, OUT_D], F32, tag="watt")  # W_att as [d, dd, j]
        nc.sync.dma_start(out=watt_sb, in_=watt.rearrange("(dd p) j -> p dd j", p=P))

        # ---- y16 load flood (SWDGE cast fp32->fp16), starts immediately ----
        CHUNKED = (0, NB - 1)
        ylist = {}
        ychunks = {}
        for k in range(NB):
            if k in CHUNKED:
                hk = hidden[k].rearrange("(p i) h -> p i h", i=TT)
                tiles = []
                for c in range(NCH):
                    yc = ybufs.tile([P, CTT, H], F16, tag="y16c", name=f"y16c_{k}_{c}")
                    nc.gpsimd.dma_start(out=yc, in_=hk[:, c * CTT : (c + 1) * CTT, :])
                    tiles.append(yc)
                ychunks[k] = tiles
            else:
                y = ybufs.tile([P, TT, H], F16, tag="y16", name=f"y16_{k}")
                nc.gpsimd.dma_start(
                    out=y, in_=hidden[k].rearrange("(p i) h -> p i h", i=TT)
                )
                ylist[k] = y

        # ---- h_t^T (fp16) and fp16 copy of W_score^T -----------------------
        htT16 = const.tile([P, 2, NB], F16, tag="htT16")  # h_t^T halves [k, half, b]
        for half in range(2):
            ps_tr = psum_t.tile([P, NB], F32, tag="ptmp", name=f"ps_tr{half}")
            nc.tensor.matmul(
                ps_tr,
                lhsT=ht_sb[:, half * P : (half + 1) * P],
                rhs=ident_sb,
                start=True,
                stop=True,
            )
            nc.scalar.copy(out=htT16[:, half, :], in_=ps_tr)
        wst16 = const.tile([P, 2, H], F16, tag="wst16")
        nc.scalar.copy(out=wst16, in_=wst_sb)

        # u[b] = h_t[b] @ W_score^T via M=1 fp16 matmuls; broadcast via a K=1
        # matmul.  No DMAs -> nothing contends with the flood.
        ubc_all = const.tile([P, NB, H], F16, tag="ubc_all")

        def emit_uchain(b):
            ps_ub = psum_u.tile([1, H], F32, tag="pub", name=f"pub{b}")
            for half in range(2):
                nc.tensor.matmul(
                    ps_ub,
                    lhsT=htT16[:, half, b : b + 1],
                    rhs=wst16[:, half, :],
                    start=(half == 0),
                    stop=(half == 1),
                )
            u16b = sc.tile([1, H], F16, tag="u16b", name=f"u16b{b}")
            nc.scalar.copy(out=u16b, in_=ps_ub)
            ps_ubc = psum_t.tile([P, H], F32, tag="ptmp", name=f"pubc{b}")
            nc.tensor.matmul(ps_ubc, lhsT=ones_row16, rhs=u16b, start=True, stop=True)
            nc.scalar.copy(out=ubc_all[:, b, :], in_=ps_ubc)

        for b in range(UPRE + 1):  # loop below starts at b=1, so chains 0..4 here
            emit_uchain(b)

        # ---- persistent PSUM accumulators: ctx^T and softmax sums ----------
        ctxT_ps = [
            psum_p.tile([P, NB], F32, tag=f"ctxT{j}", name=f"ctxT{j}")
            for j in range(2)
        ]
        s_all_ps = psum_p.tile([P, NB], F32, tag="s_all", name="s_all")

        state = {}  # batch -> dict of tiles needed by the deferred stages

        def ubc_rep(b, rep):
            ubc = ubc_all[:, b, :]
            return bass.AP(
                tensor=ubc.tensor,
                offset=ubc.offset,
                ap=[list(ubc.ap[0]), [0, rep], list(ubc.ap[1])],
            )

        def dve_tree(b, y, score_sl, nt, tag_sfx):
            z = sc.tile([P, nt, H], F16, tag="z" + tag_sfx)
            nc.vector.tensor_mul(z, y, ubc_rep(b, nt))
            z1 = sc.tile([P, nt, 128], F16, tag="z1" + tag_sfx)
            nc.vector.tensor_add(z1, z[:, :, 0:128], z[:, :, 128:256])
            z2 = sc.tile([P, nt, 64], F16, tag="z2" + tag_sfx)
            nc.vector.tensor_add(z2, z1[:, :, 0:64], z1[:, :, 64:128])
            with nc.allow_low_precision(reason="fp16 softmax scores"):
                nc.vector.tensor_reduce(
                    out=score_sl, in_=z2, axis=mybir.AxisListType.X, op=add
                )

        def emit_exp(score_sl, p_sl, q_sl):
            nc.scalar.activation(
                out=p_sl,
                in_=score_sl,
                func=mybir.ActivationFunctionType.Exp,
                bias=shift_col,
                scale=1.0,
                accum_out=q_sl,
            )

        def emit_s(b, q_sl, n):
            # q (fp32 accum) -> bf16 row(s), then a cheap bf16 ones-matmul;
            # bf16 q rounding averages out over 128 partitions (~4e-4 on s)
            q16 = sc.tile([P, n], BF16, tag="q16", name=f"q16_{b}")
            with nc.allow_low_precision(reason="softmax sum tolerates bf16 q"):
                nc.scalar.copy(out=q16, in_=q_sl)
            if n == 1:
                nc.tensor.matmul(
                    s_all_ps[:, b : b + 1],
                    lhsT=ones128_16,
                    rhs=q16,
                    start=True,
                    stop=True,
                )
            else:
                s4_ps = psum_u.tile([P, n], F32, tag="pub", name=f"s4_{b}")
                nc.tensor.matmul(s4_ps, lhsT=ones128_16, rhs=q16, start=True, stop=True)
                nc.vector.tensor_reduce(
                    out=s_all_ps[:, b : b + 1],
                    in_=s4_ps,
                    axis=mybir.AxisListType.X,
                    op=add,
                )

        def emit_ctx(b):
            p_t = state[b]["p"]
            ctx_ps = psum_t.tile([1, H], F32, tag="ptmp", name=f"ctx{b}")
            y16 = ylist.pop(b)
            for i in range(TT):
                nc.tensor.matmul(
                    ctx_ps,
                    lhsT=p_t[:, i : i + 1],
                    rhs=y16[:, i, :],
                    start=(i == 0),
                    stop=(i == TT - 1),
                )
            state[b]["ctx_ps"] = ctx_ps

        def emit_ctx_row(b):
            ctx_row = sc.tile([1, H], F32, tag="ctx_row")
            nc.scalar.copy(out=ctx_row, in_=state[b]["ctx_ps"])
            state[b]["ctx_row"] = ctx_row

        def emit_scatter(b):
            ctx_row = state[b]["ctx_row"]
            for j in range(2):
                nc.tensor.matmul(
                    ctxT_ps[j][:, b : b + 1],
                    lhsT=ctx_row[:, j * P : (j + 1) * P],
                    rhs=ones_col1,
                    start=True,
                    stop=True,
                )
            del state[b]

        def emit_chunked(b, after_first_mul=None):
            # quarter-chunk pipeline: tree/exp/ctx/s per chunk; s accumulates
            # across chunks in PSUM (bf16 p needs no pre-normalization)
            chunks = ychunks[b]
            score = sc.tile([P, TT], F16, tag="score", name=f"score{b}")
            p_t = sc.tile([P, TT], BF16, tag="p", name=f"p{b}")
            q4 = sc.tile([P, NCH], F32, tag="q4", name=f"q4_{b}")
            ctx_ps = psum_t.tile([1, H], F32, tag="ptmp", name=f"ctxc{b}")
            state[b] = {"p": p_t}
            for c in range(NCH):
                sl = slice(c * CTT, (c + 1) * CTT)
                if c == 0 and after_first_mul is not None:
                    zc = sc.tile([P, CTT, H], F16, tag="zc")
                    nc.vector.tensor_mul(zc, chunks[c], ubc_rep(b, CTT))
                    after_first_mul()
                    z1 = sc.tile([P, CTT, 128], F16, tag="z1c")
                    nc.vector.tensor_add(z1, zc[:, :, 0:128], zc[:, :, 128:256])
                    z2 = sc.tile([P, CTT, 64], F16, tag="z2c")
                    nc.vector.tensor_add(z2, z1[:, :, 0:64], z1[:, :, 64:128])
                    with nc.allow_low_precision(reason="fp16 softmax scores"):
                        nc.vector.tensor_reduce(
                            out=score[:, sl], in_=z2, axis=mybir.AxisListType.X, op=add
                        )
                else:
                    dve_tree(b, chunks[c], score[:, sl], CTT, "c")
                emit_exp(score[:, sl], p_t[:, sl], q4[:, c : c + 1])
                for i in range(c * CTT, (c + 1) * CTT):
                    nc.tensor.matmul(
                        ctx_ps,
                        lhsT=p_t[:, i : i + 1],
                        rhs=chunks[c][:, i % CTT, :],
                        start=(i == 0),
                        stop=(i == TT - 1),
                    )
            state[b]["ctx_ps"] = ctx_ps
            emit_s(b, q4, NCH)

        # ---- batch 0: chunked so the pipeline head starts ~6us earlier -----
        emit_chunked(0)

        # ---- full batches 1..14 --------------------------------------------
        for b in range(1, NB - 1):
            if b + UPRE < NB:
                emit_uchain(b + UPRE)
            score = sc.tile([P, TT], F16, tag="score")
            # mul first, then ctx(b-1) (PE) so the PE work for the previous
            # batch is emitted while the DVE chews on this batch
            z = sc.tile([P, TT, H], F16, tag="z")
            nc.vector.tensor_mul(z, ylist[b], ubc_rep(b, TT))
            if b - 1 not in CHUNKED:
                emit_ctx(b - 1)
            z1 = sc.tile([P, TT, 128], F16, tag="z1")
            nc.vector.tensor_add(z1, z[:, :, 0:128], z[:, :, 128:256])
            z2 = sc.tile([P, TT, 64], F16, tag="z2")
            nc.vector.tensor_add(z2, z1[:, :, 0:64], z1[:, :, 64:128])
            with nc.allow_low_precision(reason="fp16 softmax scores"):
                nc.vector.tensor_reduce(
                    out=score, in_=z2, axis=mybir.AxisListType.X, op=add
                )
            p_t = sc.tile([P, TT], BF16, tag="p")
            q = sc.tile([P, 1], F32, tag="q")
            state[b] = {"p": p_t}
            emit_exp(score, p_t, q)
            emit_s(b, q, 1)
            emit_ctx_row(b - 1)
            emit_scatter(b - 1)

        # ---- last batch: chunked (tail overlaps the flood) -----------------
        def _finish_b14():
            emit_ctx(NB - 2)
            emit_ctx_row(NB - 2)
            emit_scatter(NB - 2)

        emit_chunked(NB - 1, after_first_mul=_finish_b14)
        emit_ctx_row(NB - 1)
        emit_scatter(NB - 1)

        # ---- finalize: 1/s, concat with h_t, @W_att, tanh ------------------
        rs_all = sc.tile([P, NB], F32, tag="rs_all")
        nc.vector.reciprocal(out=rs_all, in_=s_all_ps)
        # watt16 cast sits here so its wait on the (slow, sync-ring) watt DMA
        # never head-of-line blocks the per-batch ACT stream
        watt16 = const.tile([P, 4, OUT_D], F16, tag="watt16")
        nc.scalar.copy(out=watt16, in_=watt_sb)
        preT = sc.tile([P, 2, NB], F16, tag="preT")
        with nc.allow_low_precision(reason="normalized ctx fits fp16"):
            for j in range(2):
                nc.vector.tensor_mul(preT[:, j, :], ctxT_ps[j], rs_all)

        out_ps = psum_t.tile([NB, OUT_D], F32, tag="ptmp")
        for dd in range(4):
            lhsT = preT[:, dd, :] if dd < 2 else htT16[:, dd - 2, :]
            nc.tensor.matmul(
                out_ps,
                lhsT=lhsT,
                rhs=watt16[:, dd, :],
                start=(dd == 0),
                stop=(dd == 3),
            )
        out_sb = sc.tile([NB, OUT_D], F32, tag="out_sb")
        nc.scalar.activation(
            out=out_sb, in_=out_ps, func=mybir.ActivationFunctionType.Tanh
        )
        nc.sync.dma_start(out=out[:, :], in_=out_sb)


_NC_CACHE = {}


def _get_nc():
    if "nc" not in _NC_CACHE:
        nc = bacc.Bacc("TRN2", target_bir_lowering=False, debug=False)
        hidden = nc.declare_dram_parameter("hidden", [NB, T, H], F32, isOutput=False)
        wst = nc.declare_dram_parameter("w_score_t", [H, H], F32, isOutput=False)
        watt = nc.declare_dram_parameter("w_att", [2 * H, OUT_D], F32, isOutput=False)
        ident = nc.declare_dram_parameter("ident16", [16, 16], F32, isOutput=False)
        out = nc.declare_dram_parameter("out", [NB, OUT_D], F32, isOutput=True)
        with tile.TileContext(nc) as tc:
            _build_kernel(nc, tc, hidden, wst, watt, ident, out)
        nc.compile()
        _NC_CACHE["nc"] = nc
    return _NC_CACHE["nc"]


def _run(hidden_states, W_score, W_att, trace=False, trace_kwargs=None):
    hidden_states = np.ascontiguousarray(np.asarray(hidden_states, dtype=np.float32))
    W_score = np.asarray(W_score, dtype=np.float32)
    W_att = np.ascontiguousarray(np.asarray(W_att, dtype=np.float32))
    wst = np.ascontiguousarray(W_score.T)
    ident = np.eye(16, dtype=np.float32)

    nc = _get_nc()
    in_maps = []
    for c in range(N_CORES):
        in_maps.append(
            {
                "hidden": hidden_states[c * NB : (c + 1) * NB],
                "w_score_t": wst,
                "w_att": W_att,
                "ident16": ident,
            }
        )
    kwargs = {}
    if trace:
        kwargs["trace"] = True
        if trace_kwargs:
            kwargs.update(trace_kwargs)
    res = run_bass_kernel_spmd(nc, in_maps, list(range(N_CORES)), **kwargs)
    out = np.concatenate([res.results[c]["out"] for c in range(N_CORES)], axis=0)
    return out, res


def kernel(hidden_states, W_score, W_att):
    out, _ = _run(hidden_states, W_score, W_att, trace=False)
    return out


# revision 29
# speedup vs baseline: 2.7306x; 1.0262x over previous
"""Trainium2 Bass kernel for nn_Attention (pooling attention).

Math (per batch b):
    u[b]     = W_score @ h_t[b]            (score = (hidden @ W_score) . h_t
                                            collapses to hidden . (W_score @ h_t))
    score[t] = hidden[b,t,:] . u[b]        (DVE fp16 mul + pairwise tree,
                                            fp16 scores)
    p[t]     = exp(score[t] - 50)          (ScalarE -> bf16, fused accum -> q)
    s[b]     = sum_t p[t]                  (PE ones-matmul of q into column b
                                            of a persistent PSUM tile)
    ctx_u    = sum_t p[t] * hidden[b,t,:]  (PE: bf16 p column as 1-col
                                            stationary vs fp16 y, UNNORMALIZED)
    ctx^T    = scatter of ctx_u rows into persistent PSUM columns
    normalize: ONE reciprocal over s_all at the end; the 1/s scale fuses into
               the ctx^T -> fp16 preT cast on DVE (per-column multiply)
    out[b]   = tanh([ctx/s, h_t[b]] @ W_att)

bf16 p is overflow-safe (fp32-range exponent), so nothing in the per-batch
chain waits on the softmax denominator; unnormalized ctx stays in fp32 PSUM
(max ~1e15 << fp32 range).  The DVE stream is a pure load-gated streak
(mul/z1/z2/reduce), with no cross-engine waits.

Sharding: data-parallel over batch, 16 batches per core on 8 cores; weights
replicated.  hidden_states is read from HBM exactly once (fp32), cast to fp16
during the DMA (SWDGE cast), and never transposed.

Pipeline design:
  - GpSimd runs ONLY the SWDGE descriptor generation (any compute op on its
    FIFO couples the score chain to buffer-WAR-blocked descgens and
    serializes the pipeline -- measured 2.5x blowup).
  - The y16 load flood starts immediately; ident/ht/wst ride the SWDGE queue
    AHEAD of the flood (they complete in ring order ~10us; anything on the
    sync ring during the flood takes 12..40us to land).  watt stays on the
    sync ring and is only casted right before the epilogue.
  - All u[b]/broadcast work happens on PE+ACT only, in fp16, interleaved with
    the batch loop.
  - First and last batches are loaded and scored in quarter-chunks so the
    pipeline head starts ~6us earlier and the tail overlaps the flood.
"""

import sys

import numpy as np

_TRN_REPO = "/opt/trn_rl_repo"
if _TRN_REPO not in sys.path:
    sys.path.insert(0, _TRN_REPO)

import concourse.bass as bass
import concourse.bacc as bacc
import concourse.tile as tile
from concourse import mybir
from concourse.bass_utils import run_bass_kernel_spmd

N_CORES = 8
B, T, H = 128, 2048, 256
NB = B // N_CORES  # batches per core
P = 128  # SBUF partitions
TT = T // P  # t-tiles per batch
OUT_D = 128
EXP_SHIFT = -50.0  # keeps exp() in fp32/bf16 range; cancels in the softmax ratio

NCH = 4  # first batch is loaded/scored in NCH chunks
CTT = TT // NCH
NCHL = 8  # last batch: finer chunks to shrink the post-flood tail
CTTL = TT // NCHL
UPRE = 4  # u-chains emitted before the loop; chain b+UPRE emitted in iter b

F32 = mybir.dt.float32
F16 = mybir.dt.float16
BF16 = mybir.dt.bfloat16


def _build_kernel(nc: bass.Bass, tc: "tile.TileContext", hidden, wst, watt, ident, out):
    add = mybir.AluOpType.add

    from contextlib import ExitStack

    with ExitStack() as ctx:
        const = ctx.enter_context(tc.tile_pool(name="const", bufs=1))
        ybufs = ctx.enter_context(tc.tile_pool(name="ybufs", bufs=12))
        sc = ctx.enter_context(tc.tile_pool(name="sc", bufs=3))
        psum_t = ctx.enter_context(tc.tile_pool(name="psum_t", bufs=3, space="PSUM"))
        psum_u = ctx.enter_context(tc.tile_pool(name="psum_u", bufs=2, space="PSUM"))
        psum_p = ctx.enter_context(tc.tile_pool(name="psum_p", bufs=1, space="PSUM"))

        # ---- constants (no DMA needed) -------------------------------------
        ones_row16 = const.tile([1, P], F16, tag="ones_row16")
        nc.vector.memset(ones_row16, 1.0)
        ones128_16 = const.tile([P, P], BF16, tag="ones128_16")
        nc.vector.memset(ones128_16, 1.0)
        ones_col1 = const.tile([1, 1], F32, tag="ones_col1")
        nc.vector.memset(ones_col1, 1.0)
        shift_col = const.tile([P, 1], F32, tag="shift_col")
        nc.vector.memset(shift_col, EXP_SHIFT)

        # ---- setup DMAs ----------------------------------------------------
        ident_sb = const.tile([16, 16], F32, tag="ident")
        nc.gpsimd.dma_start(out=ident_sb, in_=ident[:, :])
        ht_sb = const.tile([NB, H], F32, tag="ht")
        nc.gpsimd.dma_start(out=ht_sb, in_=hidden[:, T - 1, :])
        wst_sb = const.tile([P, 2, H], F32, tag="wst")  # W_score^T as [k, kk, h]
        nc.gpsimd.dma_start(out=wst_sb, in_=wst.rearrange("(kk p) h -> p kk h", p=P))
        watt_sb = const.tile([P, 4, OUT_D], F32, tag="watt")  # W_att as [d, dd, j]
        nc.sync.dma_start(out=watt_sb, in_=watt.rearrange("(dd p) j -> p dd j", p=P))

        # ---- y16 load flood (SWDGE cast fp32->fp16), starts immediately ----
        CHUNKED = (0, NB - 1)
        ylist = {}
        ychunks = {}
        for k in range(NB):
            if k in CHUNKED:
                nch = NCHL if k == NB - 1 else NCH
                ctt = TT // nch
                hk = hidden[k].rearrange("(p i) h -> p i h", i=TT)
                tiles = []
                for c in range(nch):
                    yc = ybufs.tile(
                        [P, ctt, H], F16, tag=f"y16c{ctt}", name=f"y16c_{k}_{c}"
                    )
                    nc.gpsimd.dma_start(out=yc, in_=hk[:, c * ctt : (c + 1) * ctt, :])
                    tiles.append(yc)
                ychunks[k] = tiles
            else:
                y = ybufs.tile([P, TT, H], F16, tag="y16", name=f"y16_{k}")
                nc.gpsimd.dma_start(
                    out=y, in_=hidden[k].rearrange("(p i) h -> p i h", i=TT)
                )
                ylist[k] = y

        # ---- h_t^T (fp16) and fp16 copy of W_score^T -----------------------
        htT16 = const.tile([P, 2, NB], F16, tag="htT16")  # h_t^T halves [k, half, b]
        for half in range(2):
            ps_tr = psum_t.tile([P, NB], F32, tag="ptmp", name=f"ps_tr{half}")
            nc.tensor.matmul(
                ps_tr,
                lhsT=ht_sb[:, half * P : (half + 1) * P],
                rhs=ident_sb,
                start=True,
                stop=True,
            )
            nc.scalar.copy(out=htT16[:, half, :], in_=ps_tr)
        wst16 = const.tile([P, 2, H], F16, tag="wst16")
        nc.scalar.copy(out=wst16, in_=wst_sb)

        # u[b] = h_t[b] @ W_score^T via M=1 fp16 matmuls; broadcast via a K=1
        # matmul.  No DMAs -> nothing contends with the flood.
        ubc_all = const.tile([P, NB, H], F16, tag="ubc_all")

        def emit_uchain(b):
            ps_ub = psum_u.tile([1, H], F32, tag="pub", name=f"pub{b}")
            for half in range(2):
                nc.tensor.matmul(
                    ps_ub,
                    lhsT=htT16[:, half, b : b + 1],
                    rhs=wst16[:, half, :],
                    start=(half == 0),
                    stop=(half == 1),
                )
            u16b = sc.tile([1, H], F16, tag="u16b", name=f"u16b{b}")
            nc.scalar.copy(out=u16b, in_=ps_ub)
            ps_ubc = psum_t.tile([P, H], F32, tag="ptmp", name=f"pubc{b}")
            nc.tensor.matmul(ps_ubc, lhsT=ones_row16, rhs=u16b, start=True, stop=True)
            nc.scalar.copy(out=ubc_all[:, b, :], in_=ps_ubc)

        for b in range(UPRE + 1):  # loop below starts at b=1, so chains 0..4 here
            emit_uchain(b)

        # ---- persistent PSUM accumulators: ctx^T and softmax sums ----------
        ctxT_ps = [
            psum_p.tile([P, NB], F32, tag=f"ctxT{j}", name=f"ctxT{j}")
            for j in range(2)
        ]
        s_all_ps = psum_p.tile([P, NB], F32, tag="s_all", name="s_all")

        state = {}  # batch -> dict of tiles needed by the deferred stages

        def ubc_rep(b, rep):
            ubc = ubc_all[:, b, :]
            return bass.AP(
                tensor=ubc.tensor,
                offset=ubc.offset,
                ap=[list(ubc.ap[0]), [0, rep], list(ubc.ap[1])],
            )

        def dve_tree(b, y, score_sl, nt, tag_sfx):
            z = sc.tile([P, nt, H], F16, tag="z" + tag_sfx)
            nc.vector.tensor_mul(z, y, ubc_rep(b, nt))
            z1 = sc.tile([P, nt, 128], F16, tag="z1" + tag_sfx)
            nc.vector.tensor_add(z1, z[:, :, 0:128], z[:, :, 128:256])
            z2 = sc.tile([P, nt, 64], F16, tag="z2" + tag_sfx)
            nc.vector.tensor_add(z2, z1[:, :, 0:64], z1[:, :, 64:128])
            with nc.allow_low_precision(reason="fp16 softmax scores"):
                nc.vector.tensor_reduce(
                    out=score_sl, in_=z2, axis=mybir.AxisListType.X, op=add
                )

        def emit_exp(score_sl, p_sl, q_sl):
            nc.scalar.activation(
                out=p_sl,
                in_=score_sl,
                func=mybir.ActivationFunctionType.Exp,
                bias=shift_col,
                scale=1.0,
                accum_out=q_sl,
            )

        def emit_s(b, q_sl, n):
            # q (fp32 accum) -> bf16 row(s), then a cheap bf16 ones-matmul;
            # bf16 q rounding averages out over 128 partitions (~4e-4 on s)
            q16 = sc.tile([P, n], BF16, tag="q16", name=f"q16_{b}")
            with nc.allow_low_precision(reason="softmax sum tolerates bf16 q"):
                nc.scalar.copy(out=q16, in_=q_sl)
            if n == 1:
                nc.tensor.matmul(
                    s_all_ps[:, b : b + 1],
                    lhsT=ones128_16,
                    rhs=q16,
                    start=True,
                    stop=True,
                )
            else:
                s4_ps = psum_u.tile([P, n], F32, tag="pub", name=f"s4_{b}")
                nc.tensor.matmul(s4_ps, lhsT=ones128_16, rhs=q16, start=True, stop=True)
                nc.vector.tensor_reduce(
                    out=s_all_ps[:, b : b + 1],
                    in_=s4_ps,
                    axis=mybir.AxisListType.X,
                    op=add,
                )

        def emit_ctx(b):
            p_t = state[b]["p"]
            ctx_ps = psum_t.tile([1, H], F32, tag="ptmp", name=f"ctx{b}")
            y16 = ylist.pop(b)
            for i in range(TT):
                nc.tensor.matmul(
                    ctx_ps,
                    lhsT=p_t[:, i : i + 1],
                    rhs=y16[:, i, :],
                    start=(i == 0),
                    stop=(i == TT - 1),
                )
            state[b]["ctx_ps"] = ctx_ps

        def emit_ctx_row(b):
            ctx_row = sc.tile([1, H], F32, tag="ctx_row")
            nc.scalar.copy(out=ctx_row, in_=state[b]["ctx_ps"])
            state[b]["ctx_row"] = ctx_row

        def emit_scatter(b):
            ctx_row = state[b]["ctx_row"]
            for j in range(2):
                nc.tensor.matmul(
                    ctxT_ps[j][:, b : b + 1],
                    lhsT=ctx_row[:, j * P : (j + 1) * P],
                    rhs=ones_col1,
                    start=True,
                    stop=True,
                )
            del state[b]

        def emit_chunked(b, after_first_mul=None):
            # quarter-chunk pipeline: tree/exp/ctx/s per chunk; s accumulates
            # across chunks in PSUM (bf16 p needs no pre-normalization)
            chunks = ychunks[b]
            score = sc.tile([P, TT], F16, tag="score", name=f"score{b}")
            p_t = sc.tile([P, TT], BF16, tag="p", name=f"p{b}")
            q4 = sc.tile([P, NCH], F32, tag="q4", name=f"q4_{b}")
            ctx_ps = psum_t.tile([1, H], F32, tag="ptmp", name=f"ctxc{b}")
            state[b] = {"p": p_t}
            for c in range(NCH):
                sl = slice(c * CTT, (c + 1) * CTT)
                if c == 0 and after_first_mul is not None:
                    zc = sc.tile([P, CTT, H], F16, tag="zc")
                    nc.vector.tensor_mul(zc, chunks[c], ubc_rep(b, CTT))
                    after_first_mul()
                    z1 = sc.tile([P, CTT, 128], F16, tag="z1c")
                    nc.vector.tensor_add(z1, zc[:, :, 0:128], zc[:, :, 128:256])
                    z2 = sc.tile([P, CTT, 64], F16, tag="z2c")
                    nc.vector.tensor_add(z2, z1[:, :, 0:64], z1[:, :, 64:128])
                    with nc.allow_low_precision(reason="fp16 softmax scores"):
                        nc.vector.tensor_reduce(
                            out=score[:, sl], in_=z2, axis=mybir.AxisListType.X, op=add
                        )
                else:
                    dve_tree(b, chunks[c], score[:, sl], CTT, "c")
                emit_exp(score[:, sl], p_t[:, sl], q4[:, c : c + 1])
                for i in range(c * CTT, (c + 1) * CTT):
                    nc.tensor.matmul(
                        ctx_ps,
                        lhsT=p_t[:, i : i + 1],
                        rhs=chunks[c][:, i % CTT, :],
                        start=(i == 0),
                        stop=(i == TT - 1),
                    )
            state[b]["ctx_ps"] = ctx_ps
            emit_s(b, q4, NCH)

        # ---- batch 0: chunked so the pipeline head starts ~6us earlier -----
        emit_chunked(0)

        # ---- full batches 1..14 --------------------------------------------
        for b in range(1, NB - 1):
            if b + UPRE < NB:
                emit_uchain(b + UPRE)
            score = sc.tile([P, TT], F16, tag="score")
            # mul first, then ctx(b-1) (PE) so the PE work for the previous
            # batch is emitted while the DVE chews on this batch
            z = sc.tile([P, TT, H], F16, tag="z")
            nc.vector.tensor_mul(z, ylist[b], ubc_rep(b, TT))
            if b - 1 not in CHUNKED:
                emit_ctx(b - 1)
            z1 = sc.tile([P, TT, 128], F16, tag="z1")
            nc.vector.tensor_add(z1, z[:, :, 0:128], z[:, :, 128:256])
            z2 = sc.tile([P, TT, 64], F16, tag="z2")
            nc.vector.tensor_add(z2, z1[:, :, 0:64], z1[:, :, 64:128])
            with nc.allow_low_precision(reason="fp16 softmax scores"):
                nc.vector.tensor_reduce(
                    out=score, in_=z2, axis=mybir.AxisListType.X, op=add
                )
            p_t = sc.tile([P, TT], BF16, tag="p")
            q = sc.tile([P, 1], F32, tag="q")
            state[b] = {"p": p_t}
            emit_exp(score, p_t, q)
            emit_s(b, q, 1)
            emit_ctx_row(b - 1)
            emit_scatter(b - 1)

        # ---- last batch: chunked (tail overlaps the flood) -----------------
        def _finish_b14():
            emit_ctx(NB - 2)
            emit_ctx_row(NB - 2)
            emit_scatter(NB - 2)

        emit_chunked(NB - 1, after_first_mul=_finish_b14)
        emit_ctx_row(NB - 1)
        emit_scatter(NB - 1)

        # ---- finalize: 1/s, concat with h_t, @W_att, tanh ------------------
        rs_all = sc.tile([P, NB], F32, tag="rs_all")
        nc.vector.reciprocal(out=rs_all, in_=s_all_ps)
        # watt16 cast sits here so its wait on the (slow, sync-ring) watt DMA
        # never head-of-line blocks the per-batch ACT stream
        watt16 = const.tile([P, 4, OUT_D], F16, tag="watt16")
        nc.scalar.copy(out=watt16, in_=watt_sb)
        preT = sc.tile([P, 2, NB], F16, tag="preT")
        with nc.allow_low_precision(reason="normalized ctx fits fp16"):
            for j in range(2):
                nc.vector.tensor_mul(preT[:, j, :], ctxT_ps[j], rs_all)

        out_ps = psum_t.tile([NB, OUT_D], F32, tag="ptmp")
        for dd in range(4):
            lhsT = preT[:, dd, :] if dd < 2 else htT16[:, dd - 2, :]
            nc.tensor.matmul(
                out_ps,
                lhsT=lhsT,
                rhs=watt16[:, dd, :],
                start=(dd == 0),
                stop=(dd == 3),
            )
        out_sb = sc.tile([NB, OUT_D], F32, tag="out_sb")
        nc.scalar.activation(
            out=out_sb, in_=out_ps, func=mybir.ActivationFunctionType.Tanh
        )
        nc.sync.dma_start(out=out[:, :], in_=out_sb)


_NC_CACHE = {}


def _get_nc():
    if "nc" not in _NC_CACHE:
        nc = bacc.Bacc("TRN2", target_bir_lowering=False, debug=False)
        hidden = nc.declare_dram_parameter("hidden", [NB, T, H], F32, isOutput=False)
        wst = nc.declare_dram_parameter("w_score_t", [H, H], F32, isOutput=False)
        watt = nc.declare_dram_parameter("w_att", [2 * H, OUT_D], F32, isOutput=False)
        ident = nc.declare_dram_parameter("ident16", [16, 16], F32, isOutput=False)
        out = nc.declare_dram_parameter("out", [NB, OUT_D], F32, isOutput=True)
        with tile.TileContext(nc) as tc:
            _build_kernel(nc, tc, hidden, wst, watt, ident, out)
        nc.compile()
        _NC_CACHE["nc"] = nc
    return _NC_CACHE["nc"]


def _run(hidden_states, W_score, W_att, trace=False, trace_kwargs=None):
    hidden_states = np.ascontiguousarray(np.asarray(hidden_states, dtype=np.float32))
    W_score = np.asarray(W_score, dtype=np.float32)
    W_att = np.ascontiguousarray(np.asarray(W_att, dtype=np.float32))
    wst = np.ascontiguousarray(W_score.T)
    ident = np.eye(16, dtype=np.float32)

    nc = _get_nc()
    in_maps = []
    for c in range(N_CORES):
        in_maps.append(
            {
                "hidden": hidden_states[c * NB : (c + 1) * NB],
                "w_score_t": wst,
                "w_att": W_att,
                "ident16": ident,
            }
        )
    kwargs = {}
    if trace:
        kwargs["trace"] = True
        if trace_kwargs:
            kwargs.update(trace_kwargs)
    res = run_bass_kernel_spmd(nc, in_maps, list(range(N_CORES)), **kwargs)
    out = np.concatenate([res.results[c]["out"] for c in range(N_CORES)], axis=0)
    return out, res


def kernel(hidden_states, W_score, W_att):
    out, _ = _run(hidden_states, W_score, W_att, trace=False)
    return out


# revision 31
# speedup vs baseline: 2.7646x; 1.0125x over previous
"""Trainium2 Bass kernel for nn_Attention (pooling attention).

Math (per batch b):
    u[b]     = W_score @ h_t[b]            (score = (hidden @ W_score) . h_t
                                            collapses to hidden . (W_score @ h_t))
    score[t] = hidden[b,t,:] . u[b]        (DVE fp16 mul + pairwise tree,
                                            fp16 scores)
    p[t]     = exp(score[t] - 50)          (ScalarE -> bf16, fused accum -> q)
    s[b]     = sum_t p[t]                  (PE ones-matmul of q into column b
                                            of a persistent PSUM tile)
    ctx_u    = sum_t p[t] * hidden[b,t,:]  (PE: bf16 p column as 1-col
                                            stationary vs fp16 y, UNNORMALIZED)
    ctx^T    = scatter of ctx_u rows into persistent PSUM columns
    normalize: ONE reciprocal over s_all at the end; the 1/s scale fuses into
               the ctx^T -> fp16 preT cast on DVE (per-column multiply)
    out[b]   = tanh([ctx/s, h_t[b]] @ W_att)

bf16 p is overflow-safe (fp32-range exponent), so nothing in the per-batch
chain waits on the softmax denominator; unnormalized ctx stays in fp32 PSUM
(max ~1e15 << fp32 range).  The DVE stream is a pure load-gated streak
(mul/z1/z2/reduce), with no cross-engine waits.

Sharding: data-parallel over batch, 16 batches per core on 8 cores; weights
replicated.  hidden_states is read from HBM exactly once (fp32), cast to fp16
during the DMA (SWDGE cast), and never transposed.

Pipeline design:
  - GpSimd runs ONLY the SWDGE descriptor generation (any compute op on its
    FIFO couples the score chain to buffer-WAR-blocked descgens and
    serializes the pipeline -- measured 2.5x blowup).
  - The y16 load flood starts immediately; ident/ht/wst ride the SWDGE queue
    AHEAD of the flood (they complete in ring order ~10us; anything on the
    sync ring during the flood takes 12..40us to land).  watt stays on the
    sync ring and is only casted right before the epilogue.
  - All u[b]/broadcast work happens on PE+ACT only, in fp16, interleaved with
    the batch loop.
  - First and last batches are loaded and scored in quarter-chunks so the
    pipeline head starts ~6us earlier and the tail overlaps the flood.
"""

import sys

import numpy as np

_TRN_REPO = "/opt/trn_rl_repo"
if _TRN_REPO not in sys.path:
    sys.path.insert(0, _TRN_REPO)

import concourse.bass as bass
import concourse.bacc as bacc
import concourse.tile as tile
from concourse import mybir
from concourse.bass_utils import run_bass_kernel_spmd

N_CORES = 8
B, T, H = 128, 2048, 256
NB = B // N_CORES  # batches per core
P = 128  # SBUF partitions
TT = T // P  # t-tiles per batch
OUT_D = 128
EXP_SHIFT = -50.0  # keeps exp() in fp32/bf16 range; cancels in the softmax ratio

NCH = 4  # first batch is loaded/scored in NCH chunks
CTT = TT // NCH
NCHL = 8  # last batch: finer chunks to shrink the post-flood tail
CTTL = TT // NCHL
UPRE = 4  # u-chains emitted before the loop; chain b+UPRE emitted in iter b

F32 = mybir.dt.float32
F16 = mybir.dt.float16
BF16 = mybir.dt.bfloat16


def _build_kernel(nc: bass.Bass, tc: "tile.TileContext", hidden, wst, watt, ident, out):
    add = mybir.AluOpType.add

    from contextlib import ExitStack

    with ExitStack() as ctx:
        const = ctx.enter_context(tc.tile_pool(name="const", bufs=1))
        ybufs = ctx.enter_context(tc.tile_pool(name="ybufs", bufs=10))
        sc = ctx.enter_context(tc.tile_pool(name="sc", bufs=3))
        psum_t = ctx.enter_context(tc.tile_pool(name="psum_t", bufs=3, space="PSUM"))
        psum_u = ctx.enter_context(tc.tile_pool(name="psum_u", bufs=2, space="PSUM"))
        psum_p = ctx.enter_context(tc.tile_pool(name="psum_p", bufs=1, space="PSUM"))

        # ---- constants (no DMA needed) -------------------------------------
        ones_row16 = const.tile([1, P], F16, tag="ones_row16")
        nc.vector.memset(ones_row16, 1.0)
        ones128_16 = const.tile([P, P], BF16, tag="ones128_16")
        nc.vector.memset(ones128_16, 1.0)
        ones_col1 = const.tile([1, 1], F32, tag="ones_col1")
        nc.vector.memset(ones_col1, 1.0)
        shift_col = const.tile([P, 1], F32, tag="shift_col")
        nc.vector.memset(shift_col, EXP_SHIFT)

        # ---- setup DMAs ----------------------------------------------------
        ident_sb = const.tile([16, 16], F32, tag="ident")
        nc.gpsimd.dma_start(out=ident_sb, in_=ident[:, :])
        ht_sb = const.tile([NB, H], F32, tag="ht")
        nc.gpsimd.dma_start(out=ht_sb, in_=hidden[:, T - 1, :])
        wst_sb = const.tile([P, 2, H], F32, tag="wst")  # W_score^T as [k, kk, h]
        nc.gpsimd.dma_start(out=wst_sb, in_=wst.rearrange("(kk p) h -> p kk h", p=P))
        watt_sb = const.tile([P, 4, OUT_D], F32, tag="watt")  # W_att as [d, dd, j]
        nc.sync.dma_start(out=watt_sb, in_=watt.rearrange("(dd p) j -> p dd j", p=P))

        # ---- y16 load flood (SWDGE cast fp32->fp16), starts immediately ----
        CHUNKED = (0, NB - 1)
        ylist = {}
        ychunks = {}
        for k in range(NB):
            if k in CHUNKED:
                nch = NCHL if k == NB - 1 else NCH
                ctt = TT // nch
                hk = hidden[k].rearrange("(p i) h -> p i h", i=TT)
                tiles = []
                for c in range(nch):
                    yc = ybufs.tile(
                        [P, ctt, H], F16, tag=f"y16c{ctt}", name=f"y16c_{k}_{c}"
                    )
                    nc.gpsimd.dma_start(out=yc, in_=hk[:, c * ctt : (c + 1) * ctt, :])
                    tiles.append(yc)
                ychunks[k] = tiles
            else:
                y = ybufs.tile([P, TT, H], F16, tag="y16", name=f"y16_{k}")
                nc.gpsimd.dma_start(
                    out=y, in_=hidden[k].rearrange("(p i) h -> p i h", i=TT)
                )
                ylist[k] = y

        # ---- h_t^T (fp16) and fp16 copy of W_score^T -----------------------
        htT16 = const.tile([P, 2, NB], F16, tag="htT16")  # h_t^T halves [k, half, b]
        for half in range(2):
            ps_tr = psum_t.tile([P, NB], F32, tag="ptmp", name=f"ps_tr{half}")
            nc.tensor.matmul(
                ps_tr,
                lhsT=ht_sb[:, half * P : (half + 1) * P],
                rhs=ident_sb,
                start=True,
                stop=True,
            )
            nc.scalar.copy(out=htT16[:, half, :], in_=ps_tr)
        wst16 = const.tile([P, 2, H], F16, tag="wst16")
        nc.scalar.copy(out=wst16, in_=wst_sb)

        # u[b] = h_t[b] @ W_score^T via M=1 fp16 matmuls; broadcast via a K=1
        # matmul.  No DMAs -> nothing contends with the flood.
        ubc_all = const.tile([P, NB, H], F16, tag="ubc_all")

        def emit_uchain(b):
            ps_ub = psum_u.tile([1, H], F32, tag="pub", name=f"pub{b}")
            for half in range(2):
                nc.tensor.matmul(
                    ps_ub,
                    lhsT=htT16[:, half, b : b + 1],
                    rhs=wst16[:, half, :],
                    start=(half == 0),
                    stop=(half == 1),
                )
            u16b = sc.tile([1, H], F16, tag="u16b", name=f"u16b{b}")
            nc.scalar.copy(out=u16b, in_=ps_ub)
            ps_ubc = psum_t.tile([P, H], F32, tag="ptmp", name=f"pubc{b}")
            nc.tensor.matmul(ps_ubc, lhsT=ones_row16, rhs=u16b, start=True, stop=True)
            nc.scalar.copy(out=ubc_all[:, b, :], in_=ps_ubc)

        for b in range(UPRE + 1):  # loop below starts at b=1, so chains 0..4 here
            emit_uchain(b)

        # ---- persistent PSUM accumulators: ctx^T and softmax sums ----------
        ctxT_ps = [
            psum_p.tile([P, NB], F32, tag=f"ctxT{j}", name=f"ctxT{j}")
            for j in range(2)
        ]
        s_all_ps = psum_p.tile([P, NB], F32, tag="s_all", name="s_all")

        state = {}  # batch -> dict of tiles needed by the deferred stages

        def ubc_rep(b, rep):
            ubc = ubc_all[:, b, :]
            return bass.AP(
                tensor=ubc.tensor,
                offset=ubc.offset,
                ap=[list(ubc.ap[0]), [0, rep], list(ubc.ap[1])],
            )

        def dve_tree(b, y, score_sl, nt, tag_sfx):
            z = sc.tile([P, nt, H], F16, tag="z" + tag_sfx)
            nc.vector.tensor_mul(z, y, ubc_rep(b, nt))
            z1 = sc.tile([P, nt, 128], F16, tag="z1" + tag_sfx)
            nc.vector.tensor_add(z1, z[:, :, 0:128], z[:, :, 128:256])
            z2 = sc.tile([P, nt, 64], F16, tag="z2" + tag_sfx)
            nc.vector.tensor_add(z2, z1[:, :, 0:64], z1[:, :, 64:128])
            with nc.allow_low_precision(reason="fp16 softmax scores"):
                nc.vector.tensor_reduce(
                    out=score_sl, in_=z2, axis=mybir.AxisListType.X, op=add
                )

        def emit_exp(score_sl, p_sl, q_sl):
            nc.scalar.activation(
                out=p_sl,
                in_=score_sl,
                func=mybir.ActivationFunctionType.Exp,
                bias=shift_col,
                scale=1.0,
                accum_out=q_sl,
            )

        def emit_s(b, q_sl, n):
            # q (fp32 accum) -> bf16 row(s), then a cheap bf16 ones-matmul;
            # bf16 q rounding averages out over 128 partitions (~4e-4 on s)
            q16 = sc.tile([P, n], BF16, tag="q16", name=f"q16_{b}")
            with nc.allow_low_precision(reason="softmax sum tolerates bf16 q"):
                nc.scalar.copy(out=q16, in_=q_sl)
            if n == 1:
                nc.tensor.matmul(
                    s_all_ps[:, b : b + 1],
                    lhsT=ones128_16,
                    rhs=q16,
                    start=True,
                    stop=True,
                )
            else:
                s4_ps = psum_u.tile([P, n], F32, tag="pub", name=f"s4_{b}")
                nc.tensor.matmul(s4_ps, lhsT=ones128_16, rhs=q16, start=True, stop=True)
                nc.vector.tensor_reduce(
                    out=s_all_ps[:, b : b + 1],
                    in_=s4_ps,
                    axis=mybir.AxisListType.X,
                    op=add,
                )

        def emit_ctx(b):
            p_t = state[b]["p"]
            ctx_ps = psum_t.tile([1, H], F32, tag="ptmp", name=f"ctx{b}")
            y16 = ylist.pop(b)
            for i in range(TT):
                nc.tensor.matmul(
                    ctx_ps,
                    lhsT=p_t[:, i : i + 1],
                    rhs=y16[:, i, :],
                    start=(i == 0),
                    stop=(i == TT - 1),
                )
            state[b]["ctx_ps"] = ctx_ps

        def emit_ctx_row(b):
            ctx_row = sc.tile([1, H], F32, tag="ctx_row")
            nc.scalar.copy(out=ctx_row, in_=state[b]["ctx_ps"])
            state[b]["ctx_row"] = ctx_row

        def emit_scatter(b):
            ctx_row = state[b]["ctx_row"]
            for j in range(2):
                nc.tensor.matmul(
                    ctxT_ps[j][:, b : b + 1],
                    lhsT=ctx_row[:, j * P : (j + 1) * P],
                    rhs=ones_col1,
                    start=True,
                    stop=True,
                )
            del state[b]

        def emit_chunked(b, after_first_mul=None):
            # chunked pipeline: tree/exp/ctx/s per chunk; s accumulates
            # across chunks in PSUM (bf16 p needs no pre-normalization)
            chunks = ychunks[b]
            nch = len(chunks)
            ctt = TT // nch
            tag_sfx = f"c{ctt}"
            score = sc.tile([P, TT], F16, tag="score", name=f"score{b}")
            p_t = sc.tile([P, TT], BF16, tag="p", name=f"p{b}")
            q4 = sc.tile([P, nch], F32, tag=f"q4_{nch}", name=f"q4_{b}")
            ctx_ps = psum_t.tile([1, H], F32, tag="ptmp", name=f"ctxc{b}")
            state[b] = {"p": p_t}
            for c in range(nch):
                sl = slice(c * ctt, (c + 1) * ctt)
                if c == 0 and after_first_mul is not None:
                    zc = sc.tile([P, ctt, H], F16, tag="z" + tag_sfx)
                    nc.vector.tensor_mul(zc, chunks[c], ubc_rep(b, ctt))
                    after_first_mul()
                    z1 = sc.tile([P, ctt, 128], F16, tag="z1" + tag_sfx)
                    nc.vector.tensor_add(z1, zc[:, :, 0:128], zc[:, :, 128:256])
                    z2 = sc.tile([P, ctt, 64], F16, tag="z2" + tag_sfx)
                    nc.vector.tensor_add(z2, z1[:, :, 0:64], z1[:, :, 64:128])
                    with nc.allow_low_precision(reason="fp16 softmax scores"):
                        nc.vector.tensor_reduce(
                            out=score[:, sl], in_=z2, axis=mybir.AxisListType.X, op=add
                        )
                else:
                    dve_tree(b, chunks[c], score[:, sl], ctt, tag_sfx)
                emit_exp(score[:, sl], p_t[:, sl], q4[:, c : c + 1])
                for i in range(c * ctt, (c + 1) * ctt):
                    nc.tensor.matmul(
                        ctx_ps,
                        lhsT=p_t[:, i : i + 1],
                        rhs=chunks[c][:, i % ctt, :],
                        start=(i == 0),
                        stop=(i == TT - 1),
                    )
            state[b]["ctx_ps"] = ctx_ps
            emit_s(b, q4, nch)

        # ---- batch 0: chunked so the pipeline head starts ~6us earlier -----
        emit_chunked(0)

        # ---- full batches 1..14 --------------------------------------------
        for b in range(1, NB - 1):
            if b + UPRE < NB:
                emit_uchain(b + UPRE)
            score = sc.tile([P, TT], F16, tag="score")
            # mul first, then ctx(b-1) (PE) so the PE work for the previous
            # batch is emitted while the DVE chews on this batch
            z = sc.tile([P, TT, H], F16, tag="z")
            nc.vector.tensor_mul(z, ylist[b], ubc_rep(b, TT))
            if b - 1 not in CHUNKED:
                emit_ctx(b - 1)
            z1 = sc.tile([P, TT, 128], F16, tag="z1")
            nc.vector.tensor_add(z1, z[:, :, 0:128], z[:, :, 128:256])
            z2 = sc.tile([P, TT, 64], F16, tag="z2")
            nc.vector.tensor_add(z2, z1[:, :, 0:64], z1[:, :, 64:128])
            with nc.allow_low_precision(reason="fp16 softmax scores"):
                nc.vector.tensor_reduce(
                    out=score, in_=z2, axis=mybir.AxisListType.X, op=add
                )
            p_t = sc.tile([P, TT], BF16, tag="p")
            q = sc.tile([P, 1], F32, tag="q")
            state[b] = {"p": p_t}
            emit_exp(score, p_t, q)
            emit_s(b, q, 1)
            emit_ctx_row(b - 1)
            emit_scatter(b - 1)

        # ---- last batch: chunked (tail overlaps the flood) -----------------
        def _finish_b14():
            emit_ctx(NB - 2)
            emit_ctx_row(NB - 2)
            emit_scatter(NB - 2)

        emit_chunked(NB - 1, after_first_mul=_finish_b14)
        emit_ctx_row(NB - 1)
        emit_scatter(NB - 1)

        # ---- finalize: 1/s, concat with h_t, @W_att, tanh ------------------
        rs_all = sc.tile([P, NB], F32, tag="rs_all")
        nc.vector.reciprocal(out=rs_all, in_=s_all_ps)
        # watt16 cast sits here so its wait on the (slow, sync-ring) watt DMA
        # never head-of-line blocks the per-batch ACT stream
        watt16 = const.tile([P, 4, OUT_D], F16, tag="watt16")
        nc.scalar.copy(out=watt16, in_=watt_sb)
        preT = sc.tile([P, 2, NB], F16, tag="preT")
        with nc.allow_low_precision(reason="normalized ctx fits fp16"):
            for j in range(2):
                nc.vector.tensor_mul(preT[:, j, :], ctxT_ps[j], rs_all)

        out_ps = psum_t.tile([NB, OUT_D], F32, tag="ptmp")
        for dd in range(4):
            lhsT = preT[:, dd, :] if dd < 2 else htT16[:, dd - 2, :]
            nc.tensor.matmul(
                out_ps,
                lhsT=lhsT,
                rhs=watt16[:, dd, :],
                start=(dd == 0),
                stop=(dd == 3),
            )
        out_sb = sc.tile([NB, OUT_D], F32, tag="out_sb")
        nc.scalar.activation(
            out=out_sb, in_=out_ps, func=mybir.ActivationFunctionType.Tanh
        )
        nc.sync.dma_start(out=out[:, :], in_=out_sb)


_NC_CACHE = {}


def _get_nc():
    if "nc" not in _NC_CACHE:
        nc = bacc.Bacc("TRN2", target_bir_lowering=False, debug=False)
        hidden = nc.declare_dram_parameter("hidden", [NB, T, H], F32, isOutput=False)
        wst = nc.declare_dram_parameter("w_score_t", [H, H], F32, isOutput=False)
        watt = nc.declare_dram_parameter("w_att", [2 * H, OUT_D], F32, isOutput=False)
        ident = nc.declare_dram_parameter("ident16", [16, 16], F32, isOutput=False)
        out = nc.declare_dram_parameter("out", [NB, OUT_D], F32, isOutput=True)
        with tile.TileContext(nc) as tc:
            _build_kernel(nc, tc, hidden, wst, watt, ident, out)
        nc.compile()
        _NC_CACHE["nc"] = nc
    return _NC_CACHE["nc"]


def _run(hidden_states, W_score, W_att, trace=False, trace_kwargs=None):
    hidden_states = np.ascontiguousarray(np.asarray(hidden_states, dtype=np.float32))
    W_score = np.asarray(W_score, dtype=np.float32)
    W_att = np.ascontiguousarray(np.asarray(W_att, dtype=np.float32))
    wst = np.ascontiguousarray(W_score.T)
    ident = np.eye(16, dtype=np.float32)

    nc = _get_nc()
    in_maps = []
    for c in range(N_CORES):
        in_maps.append(
            {
                "hidden": hidden_states[c * NB : (c + 1) * NB],
                "w_score_t": wst,
                "w_att": W_att,
                "ident16": ident,
            }
        )
    kwargs = {}
    if trace:
        kwargs["trace"] = True
        if trace_kwargs:
            kwargs.update(trace_kwargs)
    res = run_bass_kernel_spmd(nc, in_maps, list(range(N_CORES)), **kwargs)
    out = np.concatenate([res.results[c]["out"] for c in range(N_CORES)], axis=0)
    return out, res


def kernel(hidden_states, W_score, W_att):
    out, _ = _run(hidden_states, W_score, W_att, trace=False)
    return out
